# revision 1
# baseline (speedup 1.0000x reference)
"""GAT encoder (3-layer) on 8 Trainium2 NeuronCores.

Sharding: nodes partitioned across cores (graph partition). Edges partitioned
by destination node so segment-softmax + scatter-add stay device-local.
Weights replicated. Per-layer halo exchange = AllGather of each core's node
feature shard (transposed layout).

Device algorithm per layer (per core, rank r owns nodes [r*6272,(r+1)*6272)):
  1. H table build (all 50176 nodes, redundant on every core, avoids a 2nd
     collective): psum = h^T_tile.T @ W -> HBM table rows [Wh(128)] (512B).
  2. alpha_d for own nodes: matvec W@a_dst against own h^T, broadcast to
     [128, NLOC+64] (cols NLOC.. = -1e9 sentinel for pad tokens), then
     GpSimd indirect_copy + SBUF reshape-DMA -> per-token alpha_d.
  3. Edge phase, chunks of 2048 tokens (host guarantees each chunk has
     UNIQUE dst indices -- HW scatter-add races RMW on duplicates):
       dma_gather 512B h rows by src (single_packet=False)
       alpha_s = reduce(h * a_src) on DVE
       p = exp(leakyrelu(a_s+a_d)); payload [p*h | p | junk] (192 f32)
       dma_scatter_add into alternating out_aug buffers [NLOC+2,192]
       (cross-chunk same-buffer WAW serialization makes dups safe; the
        alternating buffer keeps the DMA pipe full; row NLOC = pad scratch)
  4. Post: h = (sum p*h)/(sum p) + b, ELU; transpose -> h^T shard; AllGather.
  Final: global_mean_pool partial sums via one-hot matmul; host combines.
"""

import math
import numpy as np

# ---------------- constants (hardcoded problem shape) ----------------
N = 50000
F = 128
G = 64
NCORES = 8
NLOC = 6272                   # 49*128 nodes per core (padded)
NPAD = NLOC * NCORES          # 50176
NTILES = NLOC // 128          # 49
TTILES = NPAD // 128          # 392
ROW = 192                     # scatter payload row width (f32) -> 768B
RTAB = NPAD + 2               # table rows; 0 = padA, RTAB-1 = padB
BANK = 32768                  # gather bank split (int16 idx range)
CHUNK = 2048
C = CHUNK // 128              # 16 tokens per partition per chunk
IC_GROUP = 2                  # chunks per indirect-copy call (ISA dst limit 512)
NAUG = NLOC + 64              # alpha_d replicated width (sentinel tail)
NEG_SLOPE = 0.2
BIG_NEG = -1.0e9
EPS = 1.0e-16
OUTROWS = NLOC + 2            # scatter dst rows (row NLOC = pad scratch)
KBUF = 2                      # scatter accumulators per layer (WAW overlap)


# ---------------- host-side preprocessing ----------------

def _assign_chunks(gs, ld, nch):
    """Assign edges to chunks s.t. each chunk has unique dst (ld).
    Round-robin per dst, staggered by dst. Returns list of (gs, ld) arrays
    per chunk, or None if some chunk overflows CHUNK."""
    order = np.argsort(ld, kind="stable")
    gs_s, ld_s = gs[order], ld[order]
    # k-th edge of its dst group
    first = np.ones(len(ld_s), bool)
    first[1:] = ld_s[1:] != ld_s[:-1]
    gidx = np.cumsum(first) - 1
    starts = np.nonzero(first)[0]
    k = np.arange(len(ld_s)) - starts[gidx]
    ch = (ld_s + k) % nch
    chunks = []
    for ci in range(nch):
        m = ch == ci
        if m.sum() > CHUNK:
            return None
        chunks.append((gs_s[m], ld_s[m]))
    return chunks


def _ic_groups(nA, nB):
    groups = []
    for bank_start, n_b in ((0, nA), (nA, nB)):
        pos = 0
        while pos < n_b:
            sz = min(IC_GROUP, n_b - pos)
            groups.append((bank_start + pos, sz))
            pos += sz
    return groups


def _build_edge_data(src, dst):
    per_core = []
    for r in range(NCORES):
        lo, hi = r * NLOC, (r + 1) * NLOC
        m = (dst >= lo) & (dst < hi)
        gs = src[m].astype(np.int64) + 1          # physical table row
        ld = (dst[m] - lo).astype(np.int64)       # local dst
        mA = gs < BANK
        per_core.append(((gs[mA], ld[mA]), (gs[~mA] - BANK, ld[~mA])))

    def n_needed(pairs):
        n = 1
        for gs, ld in pairs:
            n = max(n, int(math.ceil(len(gs) / CHUNK)))
            if len(ld):
                n = max(n, int(np.bincount(ld).max()))
        return n

    nA = n_needed([a for a, _ in per_core])
    nB = n_needed([b for _, b in per_core])

    # chunk assignment (bump n on overflow)
    assigned = None
    while assigned is None:
        assigned = []
        for r in range(NCORES):
            (gA, lA), (gB, lB) = per_core[r]
            ca = _assign_chunks(gA, lA, nA)
            cb = _assign_chunks(gB, lB, nB)
            if ca is None:
                nA += 1
                assigned = None
                break
            if cb is None:
                nB += 1
                assigned = None
                break
            assigned.append(ca + cb)

    padA_idx, padB_idx = 0, RTAB - 1 - BANK
    nCH = nA + nB
    gidx = np.zeros((NCORES, nCH, 128, CHUNK // 16), np.int16)
    sidx = np.zeros((NCORES, nCH, 128, CHUNK // 16), np.int16)
    # gidx/sidx are concatenated into one [nCH, 128, 256] input later

    t = np.arange(CHUNK)
    tr, tc = t % 16, t // 16

    # aidx: big per-bank indirect-copy streams
    # bank token array: token tt -> (p = tt%128, col = tt//128)
    # group g stream pos i = k*C_all + j ; tt = j*128 + 16g + k
    def build_aidx(ld_tok, n):
        C_all = n * C
        M = 16 * C_all
        out = np.zeros((128, M // 16), np.uint16)
        i_arr = np.arange(M)
        k_arr = i_arr // C_all
        j_arr = i_arr % C_all
        rows = i_arr % 16
        cols = i_arr // 16
        for g in range(8):
            tt = j_arr * 128 + 16 * g + k_arr
            out[16 * g + rows, cols] = ld_tok[tt].astype(np.uint16)
        return out

    groups = _ic_groups(nA, nB)

    aidx_list = []
    for r in range(NCORES):
        chunks = assigned[r]
        ld_tok = np.full(nCH * CHUNK, NLOC, np.int64)
        for ci in range(nCH):
            gs_c, ld_c = chunks[ci]
            bankB = ci >= nA
            pad = padB_idx if bankB else padA_idx
            gfull = np.full(CHUNK, pad, np.int64)
            gfull[:len(gs_c)] = gs_c
            lfull = np.zeros(CHUNK, np.int64)
            lfull[:len(ld_c)] = ld_c
            lfull[len(ld_c):] = NLOC              # pad -> scratch row
            t16 = np.zeros((16, CHUNK // 16), np.int16)
            t16[tr, tc] = gfull.astype(np.int16)
            gidx[r, ci] = np.tile(t16, (8, 1))
            s16 = np.zeros((16, CHUNK // 16), np.int16)
            s16[tr, tc] = lfull.astype(np.int16)
            sidx[r, ci] = np.tile(s16, (8, 1))
            adl = lfull.copy()
            adl[len(ld_c):] = NLOC                # pad -> -1e9 sentinel
            ld_tok[ci * CHUNK:(ci + 1) * CHUNK] = adl
        parts = [build_aidx(ld_tok[c0 * CHUNK:(c0 + gsz) * CHUNK], gsz)
                 for c0, gsz in groups]
        aidx_list.append(np.concatenate(parts, axis=1))

    aidx = np.stack(aidx_list)                    # [NCORES, 128, nCH*C]
    return gidx, sidx, aidx, nA, nB, groups


def _prep_inputs(x, edge_index, batch, Ws, asrcs, adsts, bs):
    src = np.concatenate([edge_index[0], np.arange(N, dtype=np.int64)])
    dst = np.concatenate([edge_index[1], np.arange(N, dtype=np.int64)])
    src = np.asarray(src, np.int64)
    dst = np.asarray(dst, np.int64)

    gidx, sidx, aidx, nA, nB, groups = _build_edge_data(src, dst)

    xT_full = np.zeros((F, NPAD), np.float32)
    xT_full[:, :N] = np.asarray(x, np.float32).T

    w_aug = np.zeros((3, F, F + 1), np.float32)
    for k in range(3):
        w_aug[k, :, :F] = Ws[k]
        w_aug[k, :, F] = Ws[k] @ adsts[k]

    asrc_rep = np.zeros((3, 128, F), np.float32)
    b_rep = np.zeros((3, 128, F), np.float32)
    for k in range(3):
        asrc_rep[k] = np.tile(asrcs[k][None, :], (128, 1))
        b_rep[k] = np.tile(bs[k][None, :], (128, 1))

    zrow = np.zeros((OUTROWS, ROW), np.float32)

    batch64 = np.asarray(batch, np.int64)
    phot = np.zeros((NCORES, NTILES, 128, G), np.float32)
    for r in range(NCORES):
        base = r * NLOC
        for j in range(NTILES):
            nodes = base + j * 128 + np.arange(128)
            valid = nodes < N
            gsel = batch64[np.minimum(nodes, N - 1)]
            ph = np.zeros((128, G), np.float32)
            ph[np.arange(128)[valid], gsel[valid]] = 1.0
            phot[r, j] = ph

    counts = np.bincount(batch64, minlength=G).astype(np.float32)

    in_maps = []
    for r in range(NCORES):
        in_maps.append({
            "xT_full": xT_full,
            "xT_own": np.ascontiguousarray(xT_full[:, r * NLOC:(r + 1) * NLOC]),
            "w_aug": w_aug,
            "asrc_rep": asrc_rep,
            "b_rep": b_rep,
            "zrow": zrow,
            "gsidx": np.concatenate([gidx[r], sidx[r]], axis=2),
            "aidx": aidx[r],
            "phot": phot[r].reshape(NTILES * 128, G),
        })
    return in_maps, nA, nB, counts


# ---------------- numpy emulation of the device program ----------------

def _emulate_full(in_maps, nA, nB, counts):
    nCH = nA + nB
    hT_cur = [im["xT_own"].copy() for im in in_maps]
    hT_ag = None
    pool_part = [np.zeros((G, F), np.float32) for _ in range(NCORES)]
    for k in range(3):
        new_hT = []
        for r in range(NCORES):
            im = in_maps[r]
            w = im["w_aug"][k]
            a_src = im["asrc_rep"][k][0]
            table = np.zeros((RTAB, F), np.float32)
            hsrc = im["xT_full"] if k == 0 else hT_ag
            table[1:1 + NPAD] = (hsrc.T @ w[:, :F]).astype(np.float32)
            ad_aug = np.full(NAUG, BIG_NEG, np.float32)
            ad_aug[:NLOC] = (w[:, F][None, :] @ hT_cur[r])[0]
            out_aug = np.zeros((OUTROWS, ROW), np.float32)
            for ci in range(nCH):
                bank_base = 0 if ci < nA else BANK
                g16 = im["gsidx"][ci, :, :CHUNK // 16].astype(np.int64)
                s16 = im["gsidx"][ci, :, CHUNK // 16:].astype(np.int64)
                t = np.arange(CHUNK)
                gtok = g16[t % 16, t // 16]
                stok = s16[t % 16, t // 16]
                gbuf = table[bank_base + gtok]                 # [CHUNK,128]
                # alpha_d via grouped indirect copy emulation
                groups = _ic_groups(nA, nB)
                for c0, gsz in groups:
                    if c0 <= ci < c0 + gsz:
                        break
                C_all = gsz * C
                a16 = im["aidx"][:, c0 * C:(c0 + gsz) * C].astype(np.int64)
                base_col = (ci - c0) * C
                ad_tok = np.zeros(CHUNK, np.float32)
                for g in range(8):
                    iarr = np.arange(16 * C_all)
                    stream = a16[16 * g + iarr % 16, iarr // 16]
                    kk = iarr // C_all
                    jj = iarr % C_all
                    sel = (jj >= base_col) & (jj < base_col + C)
                    tt_local = (jj[sel] - base_col) * 128 + 16 * g + kk[sel]
                    ad_tok[tt_local] = ad_aug[stream[sel]]
                al_s = gbuf @ a_src
                e = (al_s + ad_tok).astype(np.float32)
                e = np.maximum(e, NEG_SLOPE * e)
                p = np.exp(e).astype(np.float32)
                payload = np.zeros((CHUNK, ROW), np.float32)
                payload[:, :F] = gbuf * p[:, None]
                payload[:, F] = p
                np.add.at(out_aug, stok, payload)
            s = out_aug[:NLOC, F] + EPS
            h1 = (out_aug[:NLOC, :F] / s[:, None]
                  + im["b_rep"][k][0][None, :]).astype(np.float32)
            hout = np.where(h1 > 0, h1,
                            np.exp(np.minimum(h1, 0)) - 1).astype(np.float32)
            if k < 2:
                new_hT.append(hout.T.copy())
            else:
                ph = im["phot"].reshape(NTILES, 128, G)
                for j in range(NTILES):
                    pool_part[r] += ph[j].T @ hout[128 * j:128 * j + 128]
        if k < 2:
            hT_ag = np.concatenate(new_hT, axis=1)
            hT_cur = new_hT
    total = np.sum(pool_part, axis=0)
    return (total / np.maximum(counts, 1.0)[:, None]).astype(np.float32)


# ---------------- bass program ----------------

def _build_program(nA, nB, features=("gather", "ic", "scatter", "cc"),
                   repeat=1):
    import concourse.bacc as bacc
    import concourse.bass as bass
    import concourse.mybir as mybir
    import concourse.tile as tile
    from concourse import masks
    features = set(features)

    f32 = mybir.dt.float32
    i16 = mybir.dt.int16
    u16 = mybir.dt.uint16
    AF = mybir.ActivationFunctionType
    ALU = mybir.AluOpType
    AX = mybir.AxisListType
    nCH = nA + nB
    MA_COLS = nA * C          # aidx cols for bank A (per 16 rows)
    MB_COLS = nB * C

    nc = bacc.Bacc("TRN2", target_bir_lowering=False, debug=False,
                   num_devices=NCORES)

    # --- dram I/O ---
    xT_full = nc.dram_tensor("xT_full", [F, NPAD], f32, kind="ExternalInput")
    xT_own = nc.dram_tensor("xT_own", [F, NLOC], f32, kind="ExternalInput")
    w_aug_d = nc.dram_tensor("w_aug", [3, F, F + 1], f32, kind="ExternalInput")
    asrc_d = nc.dram_tensor("asrc_rep", [3, 128, F], f32, kind="ExternalInput")
    b_rep_d = nc.dram_tensor("b_rep", [3, 128, F], f32, kind="ExternalInput")
    zrow_d = nc.dram_tensor("zrow", [OUTROWS, ROW], f32, kind="ExternalInput")
    gsidx_d = nc.dram_tensor("gsidx", [nCH, 128, 2 * (CHUNK // 16)], i16,
                             kind="ExternalInput")
    aidx_d = nc.dram_tensor("aidx", [128, MA_COLS + MB_COLS], u16,
                            kind="ExternalInput")
    phot_d = nc.dram_tensor("phot", [NTILES * 128, G], f32,
                            kind="ExternalInput")
    pool_out = nc.dram_tensor("pool_part", [G, F], f32, kind="ExternalOutput")

    # --- internal dram ---
    h_table = nc.dram_tensor("h_table", [RTAB, F], f32, kind="Internal")
    out_augs = [nc.dram_tensor(f"out_aug{i}", [OUTROWS, ROW], f32,
                               kind="Internal") for i in range(3 * KBUF)]
    cc_in = nc.dram_tensor("cc_in", [F, NLOC], f32, kind="Internal")
    cc_out = nc.dram_tensor("cc_out", [NCORES, F, NLOC], f32, kind="Internal",
                            addr_space="Shared")

    with tile.TileContext(nc) as tc:
        with (
            tc.tile_pool(name="persist", bufs=1) as persist,
            tc.tile_pool(name="lhs", bufs=4) as lhs_pool,
            tc.tile_pool(name="stage", bufs=4) as stage_pool,
            tc.tile_pool(name="edge", bufs=3) as edge_pool,
            tc.tile_pool(name="gb", bufs=2) as gb_pool,
            tc.tile_pool(name="post", bufs=3) as post_pool,
            tc.tile_pool(name="ps", bufs=2, space="PSUM") as ps_pool,
            tc.tile_pool(name="pstr", bufs=2, space="PSUM") as pstr_pool,
            tc.tile_pool(name="ps1", bufs=1, space="PSUM") as ps1_pool,
            tc.tile_pool(name="psb", bufs=1, space="PSUM") as psb_pool,
            tc.tile_pool(name="pspool", bufs=1, space="PSUM") as pspool_pool,
        ):
            # persistent tiles
            hT = persist.tile([F, NLOC], f32, tag="hT")
            ad_rep = persist.tile([128, NAUG], f32, tag="ad_rep")
            ad_row = persist.tile([1, NLOC], f32, tag="ad_row")
            adt_all = persist.tile([128, nCH * C], f32, tag="adt_all")
            identity = persist.tile([128, 128], f32, tag="identity")
            ones_col = persist.tile([1, 128], f32, tag="ones_col")
            w_sb = persist.tile([F, F + 1], f32, tag="w_sb")
            asrc_sb = persist.tile([128, F], f32, tag="asrc_sb")
            b_sb = persist.tile([128, F], f32, tag="b_sb")
            ic_out = persist.tile([128, 16 * IC_GROUP * C], f32,
                                  tag="ic_out")
            pay_bufs = [persist.tile([128, C, ROW], f32, tag=f"pay{i}",
                                     name=f"pay{i}")
                        for i in range(KBUF)]
            aidx_sb = persist.tile([128, MA_COLS + MB_COLS], u16,
                                   tag="aidx_sb")

            masks.make_identity(nc, identity[:])
            nc.gpsimd.memset(ones_col[:], 1.0)
            nc.sync.dma_start(aidx_sb[:], aidx_d.ap())
            # zero pad rows of the gather table
            zpad = persist.tile([2, F], f32, tag="zpad")
            nc.gpsimd.memset(zpad[:], 0.0)
            nc.sync.dma_start(h_table.ap()[0:1], zpad[0:1])
            nc.sync.dma_start(h_table.ap()[RTAB - 1:RTAB], zpad[1:2])

            for pb_ in pay_bufs:
                nc.vector.memset(pb_[:, :, F + 1:ROW], 0.0)
            for rep in range(repeat):
              nc.sync.dma_start(hT[:], xT_own.ap())
              for oa in out_augs:
                nc.scalar.dma_start(oa.ap()[:], zrow_d.ap()[:])
              for k in range(3):
                  nc.sync.dma_start(w_sb[:], w_aug_d.ap()[k])
                  nc.sync.dma_start(asrc_sb[:], asrc_d.ap()[k])
                  nc.sync.dma_start(b_sb[:], b_rep_d.ap()[k])

                  # ---- table build: all NPAD nodes, blocked ----
                  # block loads/stores cut HWDGE instruction count; loads on
                  # SP queue, stores on ACT queue to parallelize sequencers
                  def table_block(t0, nt, load_src):
                      lhsT = lhs_pool.tile([128, 4, 128], f32, tag="lhsT")
                      load_src(lhsT, t0, nt)
                      ps = ps_pool.tile([128, 4, F], f32, tag="ps_tab")
                      for i in range(nt):
                          nc.tensor.matmul(ps[:, i], lhsT[:, i], w_sb[:, 0:F],
                                           start=True, stop=True)
                      st = stage_pool.tile([128, 4, F], f32, tag="stage")
                      nc.scalar.activation(st[:, 0:nt], ps[:, 0:nt], AF.Copy)
                      dst = h_table.ap()[1 + 128 * t0:1 + 128 * (t0 + nt)] \
                          .rearrange("(t p) f -> p t f", t=nt)
                      nc.scalar.dma_start(dst, st[:, 0:nt])

                  if "notable" in features:
                      pass
                  elif k == 0:
                      def load0(lhsT, t0, nt):
                          nc.sync.dma_start(
                              lhsT[:, 0:nt],
                              xT_full.ap()[:, 128 * t0:128 * (t0 + nt)]
                              .rearrange("p (t f) -> p t f", t=nt))
                      for blk in range(TTILES // 4):
                          table_block(4 * blk, 4, load0)
                  else:
                      def load1(lhsT, t0, nt):
                          rr, jj = t0 // NTILES, t0 % NTILES
                          nc.sync.dma_start(
                              lhsT[:, 0:nt],
                              cc_out.ap()[rr, :, 128 * jj:128 * (jj + nt)]
                              .rearrange("p (t f) -> p t f", t=nt))
                      for rr in range(NCORES):
                          base = rr * NTILES
                          pos = 0
                          while pos < NTILES:
                              nt = min(4, NTILES - pos)
                              table_block(base + pos, nt, load1)
                              pos += nt

                  # ---- alpha_d of own nodes -> replicated [128, NAUG] ----
                  ad_chunks = []
                  pos = 0
                  while pos < NLOC:
                      sz = min(512, NLOC - pos)
                      ad_chunks.append((pos, sz))
                      pos += sz
                  for pos, sz in ad_chunks:
                      sl = slice(pos, pos + sz)
                      pr = ps1_pool.tile([1, 512], f32, tag="ps_ad")
                      nc.tensor.matmul(pr[:, 0:sz], w_sb[:, F:F + 1],
                                       hT[:, sl], start=True, stop=True)
                      nc.vector.tensor_copy(ad_row[:, sl], pr[:, 0:sz])
                  for pos, sz in ad_chunks:
                      sl = slice(pos, pos + sz)
                      pb = psb_pool.tile([128, 512], f32, tag="ps_bc")
                      nc.tensor.matmul(pb[:, 0:sz], ones_col[:], ad_row[:, sl],
                                       start=True, stop=True)
                      nc.vector.tensor_copy(ad_rep[:, sl], pb[:, 0:sz])
                  nc.vector.memset(ad_rep[:, NLOC:NAUG], BIG_NEG)

                  # ---- per-token alpha_d: 2 big indirect copies + reshape ----
                  if "ic" in features:
                      for c0, gsz in _ic_groups(nA, nB):
                          C_all = gsz * C
                          M = 16 * C_all
                          nc.gpsimd.indirect_copy(
                              ic_out[:, 0:M], ad_rep[:],
                              aidx_sb[:, c0 * C:c0 * C + C_all], True)
                          src_ap = ic_out[:, 0:M].rearrange(
                              "(g o) (kk j) -> g o kk j",
                              g=8, o=16, kk=16, j=C_all)[:, 0]
                          nc.sync.dma_start(
                              adt_all[:, c0 * C:c0 * C + C_all], src_ap)
                  else:
                      nc.vector.memset(adt_all[:], 0.0)

                  # ---- edge phase ----
                  for ci in range(nCH):
                      bank = h_table.ap()[0:BANK] if ci < nA \
                          else h_table.ap()[BANK:RTAB]
                      gsi = edge_pool.tile([128, 2 * (CHUNK // 16)], i16,
                                           tag="gsi")
                      nc.sync.dma_start(gsi[:], gsidx_d.ap()[ci])
                      gi = gsi[:, 0:CHUNK // 16]
                      si = gsi[:, CHUNK // 16:2 * (CHUNK // 16)]

                      gbuf = gb_pool.tile([128, C, F], f32, tag="gbuf")
                      if "gather" in features:
                          nc.gpsimd.dma_gather(gbuf[:], bank, gi, CHUNK,
                                               CHUNK, F, single_packet=False)
                      else:
                          nc.vector.memset(gbuf[:], 0.0)

                      als = edge_pool.tile([128, C], f32, tag="als")
                      if "noedve" in features:
                          nc.vector.memset(als[:], 0.0)
                      else:
                          prod = edge_pool.tile([128, C, F], f32, tag="prod")
                          a_bc = asrc_sb[:].unsqueeze(1).broadcast_to([128, C, F])
                          nc.vector.tensor_tensor(prod[:], gbuf[:], a_bc,
                                                  ALU.mult)
                          nc.vector.tensor_reduce(als[:], prod[:], AX.X, ALU.add)

                      e = edge_pool.tile([128, C], f32, tag="e")
                      nc.vector.tensor_tensor(e[:], als[:],
                                              adt_all[:, ci * C:ci * C + C],
                                              ALU.add)
                      nc.vector.scalar_tensor_tensor(e[:], e[:], NEG_SLOPE, e[:],
                                                     ALU.mult, ALU.max)
                      p = edge_pool.tile([128, C], f32, tag="p")
                      nc.scalar.activation(p[:], e[:], AF.Exp)

                      pay = pay_bufs[ci % KBUF]
                      if "noedve" not in features:
                          p_b = p[:].unsqueeze(2).broadcast_to([128, C, F])
                          nc.vector.tensor_tensor(pay[:, :, 0:F], gbuf[:], p_b,
                                                  ALU.mult)
                          nc.vector.tensor_copy(pay[:, :, F], p[:])
                      if "scatter" in features:
                          nc.gpsimd.dma_scatter_add(
                              out_augs[2 * k + ci % 2].ap()[:], pay[:], si,
                              CHUNK, CHUNK, ROW, single_packet=False)

                  # ---- post-process ----
                  if k == 2:
                      ps_pl = pspool_pool.tile([G, F], f32, tag="ps_pl")
                  for j in range(NTILES):
                      poA = post_pool.tile([128, F + 1], f32, tag="poA")
                      poB = post_pool.tile([128, F + 1], f32, tag="poB")
                      nc.scalar.dma_start(
                          poA[:],
                          out_augs[2 * k].ap()[128 * j:128 * j + 128, 0:F + 1])
                      nc.scalar.dma_start(
                          poB[:],
                          out_augs[2 * k + 1].ap()[128 * j:128 * j + 128, 0:F + 1])
                      po = post_pool.tile([128, F + 1], f32, tag="po")
                      nc.vector.tensor_tensor(po[:], poA[:], poB[:], ALU.add)
                      s_t = post_pool.tile([128, 1], f32, tag="s_t")
                      nc.vector.tensor_scalar_add(s_t[:], po[:, F:F + 1], EPS)
                      r_t = post_pool.tile([128, 1], f32, tag="r_t")
                      nc.vector.reciprocal(r_t[:], s_t[:])
                      h1 = post_pool.tile([128, F], f32, tag="h1")
                      nc.vector.tensor_scalar(h1[:], po[:, 0:F], r_t[:], None,
                                              ALU.mult)
                      nc.vector.tensor_tensor(h1[:], h1[:], b_sb[:], ALU.add)
                      # ELU = relu(x) + expm1(min(x,0))
                      mn = post_pool.tile([128, F], f32, tag="mn")
                      nc.vector.tensor_scalar_min(mn[:], h1[:], 0.0)
                      ex = post_pool.tile([128, F], f32, tag="ex")
                      nc.scalar.activation(ex[:], mn[:], AF.Exp)
                      rl = post_pool.tile([128, F], f32, tag="rl")
                      nc.vector.tensor_scalar_max(rl[:], h1[:], 0.0)
                      ho = post_pool.tile([128, F], f32, tag="ho")
                      nc.vector.scalar_tensor_tensor(ho[:], ex[:], -1.0, rl[:],
                                                     ALU.add, ALU.add)
                      if k < 2:
                          pt = pstr_pool.tile([128, 128], f32, tag="ps_tr")
                          nc.tensor.transpose(pt[:], ho[:], identity[:])
                          nc.vector.tensor_copy(hT[:, 128 * j:128 * j + 128],
                                                pt[:])
                      else:
                          ph = post_pool.tile([128, G], f32, tag="ph")
                          nc.sync.dma_start(
                              ph[:], phot_d.ap()[128 * j:128 * j + 128])
                          nc.tensor.matmul(ps_pl[:], ph[:], ho[:],
                                           start=(j == 0),
                                           stop=(j == NTILES - 1))

                  if k < 2:
                      nc.sync.dma_start(cc_in.ap()[:], hT[:])
                      if "cc" in features:
                          nc.gpsimd.collective_compute(
                              "AllGather", mybir.AluOpType.bypass,
                              replica_groups=[list(range(NCORES))],
                              ins=[cc_in.ap().opt()], outs=[cc_out.ap().opt()])
                      else:
                          for rr in range(NCORES):
                              nc.sync.dma_start(cc_out.ap()[rr], cc_in.ap()[:])
                  else:
                      pl_sb = post_pool.tile([G, F], f32, tag="pl_sb")
                      nc.vector.tensor_copy(pl_sb[:], ps_pl[:])
                      nc.sync.dma_start(pool_out.ap()[:], pl_sb[:])

    nc.compile()
    return nc


# ---------------- entry point ----------------

LAST_EXEC_NS = None


def kernel(x, edge_index, batch,
           W1, a_src1, a_dst1, b1,
           W2, a_src2, a_dst2, b2,
           W3, a_src3, a_dst3, b3):
    global LAST_EXEC_NS
    x = np.asarray(x, np.float32)
    edge_index = np.asarray(edge_index)
    batch = np.asarray(batch)
    Ws = [np.asarray(W1, np.float32), np.asarray(W2, np.float32),
          np.asarray(W3, np.float32)]
    asrcs = [np.asarray(a_src1, np.float32), np.asarray(a_src2, np.float32),
             np.asarray(a_src3, np.float32)]
    adsts = [np.asarray(a_dst1, np.float32), np.asarray(a_dst2, np.float32),
             np.asarray(a_dst3, np.float32)]
    bs = [np.asarray(b1, np.float32), np.asarray(b2, np.float32),
          np.asarray(b3, np.float32)]

    in_maps, nA, nB, counts = _prep_inputs(x, edge_index, batch, Ws, asrcs,
                                           adsts, bs)

    from concourse.bass_utils import run_bass_kernel_spmd
    nc = _build_program(nA, nB)
    res = run_bass_kernel_spmd(nc, in_maps, core_ids=list(range(NCORES)))
    LAST_EXEC_NS = res.exec_time_ns
    total = np.zeros((G, F), np.float32)
    for r in range(NCORES):
        total += res.results[r]["pool_part"]
    out = total / np.maximum(counts, 1.0)[:, None]
    return out.astype(np.float32)



# revision 4
# speedup vs baseline: 1.9697x; 1.9697x over previous
"""GAT encoder (3-layer) on 8 Trainium2 NeuronCores.

Sharding: nodes partitioned across cores (graph partition). Edges partitioned
by destination node so segment-softmax + scatter-add stay device-local.
Weights replicated. Per-layer halo exchange = AllGather of each core's node
feature shard (transposed layout).

Device algorithm per layer (per core, rank r owns nodes [r*6272,(r+1)*6272)):
  1. H table build (all 50176 nodes, redundant on every core, avoids a 2nd
     collective): psum = h^T_tile.T @ W -> HBM table rows [Wh(128)] (512B).
  2. alpha_d for own nodes: matvec W@a_dst against own h^T, broadcast to
     [128, NLOC+64] (cols NLOC.. = -1e9 sentinel for pad tokens), then
     GpSimd indirect_copy + SBUF reshape-DMA -> per-token alpha_d.
  3. Edge phase, chunks of 2048 tokens (host guarantees each chunk has
     UNIQUE dst indices -- HW scatter-add races RMW on duplicates):
       dma_gather 512B h rows by src (single_packet=False)
       alpha_s = reduce(h * a_src) on DVE
       p = exp(leakyrelu(a_s+a_d)); payload [p*h | p | junk] (192 f32)
       dma_scatter_add into alternating out_aug buffers [NLOC+2,192]
       (cross-chunk same-buffer WAW serialization makes dups safe; the
        alternating buffer keeps the DMA pipe full; row NLOC = pad scratch)
  4. Post: h = (sum p*h)/(sum p) + b, ELU; transpose -> h^T shard; AllGather.
  Final: global_mean_pool partial sums via one-hot matmul; host combines.
"""

import math
import numpy as np

# ---------------- constants (hardcoded problem shape) ----------------
N = 50000
F = 128
G = 64
NCORES = 8
NLOC = 6272                   # 49*128 nodes per core (padded)
NPAD = NLOC * NCORES          # 50176
NTILES = NLOC // 128          # 49
TTILES = NPAD // 128          # 392
ROW = 192                     # scatter payload row width (f32) -> 768B
RTAB = NPAD + 2               # table rows; 0 = padA, RTAB-1 = padB
BANK = 32768                  # gather bank split (int16 idx range)
CHUNK = 2048
C = CHUNK // 128              # 16 tokens per partition per chunk
IC_GROUP = 2                  # chunks per indirect-copy call (ISA dst limit 512)
NAUG = NLOC + 64              # alpha_d replicated width (sentinel tail)
NEG_SLOPE = 0.2
BIG_NEG = -1.0e9
EPS = 1.0e-16
OUTROWS = NLOC + 2            # scatter dst rows (row NLOC = pad scratch)
KBUF = 2                      # scatter accumulators per layer (WAW overlap)


# ---------------- host-side preprocessing ----------------

def _assign_chunks(gs, ld, nch):
    """Assign edges to chunks s.t. each chunk has unique dst (ld).
    Round-robin per dst, staggered by dst. Returns list of (gs, ld) arrays
    per chunk, or None if some chunk overflows CHUNK."""
    order = np.argsort(ld, kind="stable")
    gs_s, ld_s = gs[order], ld[order]
    # k-th edge of its dst group
    first = np.ones(len(ld_s), bool)
    first[1:] = ld_s[1:] != ld_s[:-1]
    gidx = np.cumsum(first) - 1
    starts = np.nonzero(first)[0]
    k = np.arange(len(ld_s)) - starts[gidx]
    ch = (ld_s + k) % nch
    chunks = []
    for ci in range(nch):
        m = ch == ci
        if m.sum() > CHUNK:
            return None
        chunks.append((gs_s[m], ld_s[m]))
    return chunks


def _ic_groups(nA, nB):
    groups = []
    for bank_start, n_b in ((0, nA), (nA, nB)):
        pos = 0
        while pos < n_b:
            sz = min(IC_GROUP, n_b - pos)
            groups.append((bank_start + pos, sz))
            pos += sz
    return groups


def _build_edge_data(src, dst):
    per_core = []
    for r in range(NCORES):
        lo, hi = r * NLOC, (r + 1) * NLOC
        m = (dst >= lo) & (dst < hi)
        gs = src[m].astype(np.int64) + 1          # physical table row
        ld = (dst[m] - lo).astype(np.int64)       # local dst
        mA = gs < BANK
        per_core.append(((gs[mA], ld[mA]), (gs[~mA] - BANK, ld[~mA])))

    def n_needed(pairs):
        n = 1
        for gs, ld in pairs:
            n = max(n, int(math.ceil(len(gs) / CHUNK)))
            if len(ld):
                n = max(n, int(np.bincount(ld).max()))
        return n

    nA = n_needed([a for a, _ in per_core])
    nB = n_needed([b for _, b in per_core])

    # chunk assignment (bump n on overflow)
    assigned = None
    while assigned is None:
        assigned = []
        for r in range(NCORES):
            (gA, lA), (gB, lB) = per_core[r]
            ca = _assign_chunks(gA, lA, nA)
            cb = _assign_chunks(gB, lB, nB)
            if ca is None:
                nA += 1
                assigned = None
                break
            if cb is None:
                nB += 1
                assigned = None
                break
            assigned.append(ca + cb)

    padA_idx, padB_idx = 0, RTAB - 1 - BANK
    nCH = nA + nB
    gidx = np.zeros((NCORES, nCH, 128, CHUNK // 16), np.int16)
    sidx = np.zeros((NCORES, nCH, 128, CHUNK // 16), np.int16)
    # gidx/sidx are concatenated into one [nCH, 128, 256] input later

    t = np.arange(CHUNK)
    tr, tc = t % 16, t // 16

    # aidx: big per-bank indirect-copy streams
    # bank token array: token tt -> (p = tt%128, col = tt//128)
    # group g stream pos i = k*C_all + j ; tt = j*128 + 16g + k
    def build_aidx(ld_tok, n):
        C_all = n * C
        M = 16 * C_all
        out = np.zeros((128, M // 16), np.uint16)
        i_arr = np.arange(M)
        k_arr = i_arr // C_all
        j_arr = i_arr % C_all
        rows = i_arr % 16
        cols = i_arr // 16
        for g in range(8):
            tt = j_arr * 128 + 16 * g + k_arr
            out[16 * g + rows, cols] = ld_tok[tt].astype(np.uint16)
        return out

    groups = _ic_groups(nA, nB)

    aidx_list = []
    for r in range(NCORES):
        chunks = assigned[r]
        ld_tok = np.full(nCH * CHUNK, NLOC, np.int64)
        for ci in range(nCH):
            gs_c, ld_c = chunks[ci]
            bankB = ci >= nA
            pad = padB_idx if bankB else padA_idx
            gfull = np.full(CHUNK, pad, np.int64)
            gfull[:len(gs_c)] = gs_c
            lfull = np.zeros(CHUNK, np.int64)
            lfull[:len(ld_c)] = ld_c
            lfull[len(ld_c):] = NLOC              # pad -> scratch row
            t16 = np.zeros((16, CHUNK // 16), np.int16)
            t16[tr, tc] = gfull.astype(np.int16)
            gidx[r, ci] = np.tile(t16, (8, 1))
            s16 = np.zeros((16, CHUNK // 16), np.int16)
            s16[tr, tc] = lfull.astype(np.int16)
            sidx[r, ci] = np.tile(s16, (8, 1))
            adl = lfull.copy()
            adl[len(ld_c):] = NLOC                # pad -> -1e9 sentinel
            ld_tok[ci * CHUNK:(ci + 1) * CHUNK] = adl
        parts = [build_aidx(ld_tok[c0 * CHUNK:(c0 + gsz) * CHUNK], gsz)
                 for c0, gsz in groups]
        aidx_list.append(np.concatenate(parts, axis=1))

    aidx = np.stack(aidx_list)                    # [NCORES, 128, nCH*C]
    return gidx, sidx, aidx, nA, nB, groups


def _prep_inputs(x, edge_index, batch, Ws, asrcs, adsts, bs):
    src = np.concatenate([edge_index[0], np.arange(N, dtype=np.int64)])
    dst = np.concatenate([edge_index[1], np.arange(N, dtype=np.int64)])
    src = np.asarray(src, np.int64)
    dst = np.asarray(dst, np.int64)

    gidx, sidx, aidx, nA, nB, groups = _build_edge_data(src, dst)

    xT_full = np.zeros((F, NPAD), np.float32)
    xT_full[:, :N] = np.asarray(x, np.float32).T

    w_aug = np.zeros((3, F, F + 1), np.float32)
    for k in range(3):
        w_aug[k, :, :F] = Ws[k]
        w_aug[k, :, F] = Ws[k] @ adsts[k]

    asrc_rep = np.zeros((3, 128, F), np.float32)
    b_rep = np.zeros((3, 128, F), np.float32)
    for k in range(3):
        asrc_rep[k] = np.tile(asrcs[k][None, :], (128, 1))
        b_rep[k] = np.tile(bs[k][None, :], (128, 1))

    zrow = np.zeros((OUTROWS, ROW), np.float32)

    batch64 = np.asarray(batch, np.int64)
    phot = np.zeros((NCORES, NTILES, 128, G), np.float32)
    for r in range(NCORES):
        base = r * NLOC
        for j in range(NTILES):
            nodes = base + j * 128 + np.arange(128)
            valid = nodes < N
            gsel = batch64[np.minimum(nodes, N - 1)]
            ph = np.zeros((128, G), np.float32)
            ph[np.arange(128)[valid], gsel[valid]] = 1.0
            phot[r, j] = ph

    counts = np.bincount(batch64, minlength=G).astype(np.float32)

    in_maps = []
    for r in range(NCORES):
        in_maps.append({
            "xT_full": xT_full,
            "xT_own": np.ascontiguousarray(xT_full[:, r * NLOC:(r + 1) * NLOC]),
            "w_aug": w_aug,
            "asrc_rep": asrc_rep,
            "b_rep": b_rep,
            "zrow": zrow,
            "gsidx": np.concatenate([gidx[r], sidx[r]], axis=2),
            "aidx": aidx[r],
            "phot": phot[r].reshape(NTILES * 128, G),
        })
    return in_maps, nA, nB, counts


# ---------------- numpy emulation of the device program ----------------

def _emulate_full(in_maps, nA, nB, counts):
    nCH = nA + nB
    hT_cur = [im["xT_own"].copy() for im in in_maps]
    hT_ag = None
    pool_part = [np.zeros((G, F), np.float32) for _ in range(NCORES)]
    for k in range(3):
        new_hT = []
        for r in range(NCORES):
            im = in_maps[r]
            w = im["w_aug"][k]
            a_src = im["asrc_rep"][k][0]
            table = np.zeros((RTAB, F), np.float32)
            hsrc = im["xT_full"] if k == 0 else hT_ag
            table[1:1 + NPAD] = (hsrc.T @ w[:, :F]).astype(np.float32)
            ad_aug = np.full(NAUG, BIG_NEG, np.float32)
            ad_aug[:NLOC] = (w[:, F][None, :] @ hT_cur[r])[0]
            out_aug = np.zeros((OUTROWS, ROW), np.float32)
            for ci in range(nCH):
                bank_base = 0 if ci < nA else BANK
                g16 = im["gsidx"][ci, :, :CHUNK // 16].astype(np.int64)
                s16 = im["gsidx"][ci, :, CHUNK // 16:].astype(np.int64)
                t = np.arange(CHUNK)
                gtok = g16[t % 16, t // 16]
                stok = s16[t % 16, t // 16]
                gbuf = table[bank_base + gtok]                 # [CHUNK,128]
                # alpha_d via grouped indirect copy emulation
                groups = _ic_groups(nA, nB)
                for c0, gsz in groups:
                    if c0 <= ci < c0 + gsz:
                        break
                C_all = gsz * C
                a16 = im["aidx"][:, c0 * C:(c0 + gsz) * C].astype(np.int64)
                base_col = (ci - c0) * C
                ad_tok = np.zeros(CHUNK, np.float32)
                for g in range(8):
                    iarr = np.arange(16 * C_all)
                    stream = a16[16 * g + iarr % 16, iarr // 16]
                    kk = iarr // C_all
                    jj = iarr % C_all
                    sel = (jj >= base_col) & (jj < base_col + C)
                    tt_local = (jj[sel] - base_col) * 128 + 16 * g + kk[sel]
                    ad_tok[tt_local] = ad_aug[stream[sel]]
                al_s = gbuf @ a_src
                e = (al_s + ad_tok).astype(np.float32)
                e = np.maximum(e, NEG_SLOPE * e)
                p = np.exp(e).astype(np.float32)
                payload = np.zeros((CHUNK, ROW), np.float32)
                payload[:, :F] = gbuf * p[:, None]
                payload[:, F] = p
                np.add.at(out_aug, stok, payload)
            s = out_aug[:NLOC, F] + EPS
            h1 = (out_aug[:NLOC, :F] / s[:, None]
                  + im["b_rep"][k][0][None, :]).astype(np.float32)
            hout = np.where(h1 > 0, h1,
                            np.exp(np.minimum(h1, 0)) - 1).astype(np.float32)
            if k < 2:
                new_hT.append(hout.T.copy())
            else:
                ph = im["phot"].reshape(NTILES, 128, G)
                for j in range(NTILES):
                    pool_part[r] += ph[j].T @ hout[128 * j:128 * j + 128]
        if k < 2:
            hT_ag = np.concatenate(new_hT, axis=1)
            hT_cur = new_hT
    total = np.sum(pool_part, axis=0)
    return (total / np.maximum(counts, 1.0)[:, None]).astype(np.float32)


# ---------------- bass program ----------------

def _build_program(nA, nB, features=("gather", "ic", "scatter", "cc"),
                   repeat=1):
    import concourse.bacc as bacc
    import concourse.bass as bass
    import concourse.mybir as mybir
    import concourse.tile as tile
    from concourse import masks
    features = set(features)

    f32 = mybir.dt.float32
    i16 = mybir.dt.int16
    u16 = mybir.dt.uint16
    AF = mybir.ActivationFunctionType
    ALU = mybir.AluOpType
    AX = mybir.AxisListType
    nCH = nA + nB
    MA_COLS = nA * C          # aidx cols for bank A (per 16 rows)
    MB_COLS = nB * C

    nc = bacc.Bacc("TRN2", target_bir_lowering=False, debug=False,
                   num_devices=NCORES)

    # --- dram I/O ---
    xT_full = nc.dram_tensor("xT_full", [F, NPAD], f32, kind="ExternalInput")
    xT_own = nc.dram_tensor("xT_own", [F, NLOC], f32, kind="ExternalInput")
    w_aug_d = nc.dram_tensor("w_aug", [3, F, F + 1], f32, kind="ExternalInput")
    asrc_d = nc.dram_tensor("asrc_rep", [3, 128, F], f32, kind="ExternalInput")
    b_rep_d = nc.dram_tensor("b_rep", [3, 128, F], f32, kind="ExternalInput")
    zrow_d = nc.dram_tensor("zrow", [OUTROWS, ROW], f32, kind="ExternalInput")
    gsidx_d = nc.dram_tensor("gsidx", [nCH, 128, 2 * (CHUNK // 16)], i16,
                             kind="ExternalInput")
    aidx_d = nc.dram_tensor("aidx", [128, MA_COLS + MB_COLS], u16,
                            kind="ExternalInput")
    phot_d = nc.dram_tensor("phot", [NTILES * 128, G], f32,
                            kind="ExternalInput")
    pool_out = nc.dram_tensor("pool_part", [G, F], f32, kind="ExternalOutput")

    # --- internal dram ---
    h_table = nc.dram_tensor("h_table", [RTAB, F], f32, kind="Internal")
    out_augs = [nc.dram_tensor(f"out_aug{i}", [OUTROWS, ROW], f32,
                               kind="Internal") for i in range(3 * KBUF)]
    cc_in = nc.dram_tensor("cc_in", [F, NLOC], f32, kind="Internal")
    cc_out = nc.dram_tensor("cc_out", [NCORES, F, NLOC], f32, kind="Internal",
                            addr_space="Shared")

    with tile.TileContext(nc) as tc:
        with (
            tc.tile_pool(name="persist", bufs=1) as persist,
            tc.tile_pool(name="lhs", bufs=4) as lhs_pool,
            tc.tile_pool(name="stage", bufs=4) as stage_pool,
            tc.tile_pool(name="edge", bufs=3) as edge_pool,
            tc.tile_pool(name="gb", bufs=2) as gb_pool,
            tc.tile_pool(name="post", bufs=3) as post_pool,
            tc.tile_pool(name="ps", bufs=2, space="PSUM") as ps_pool,
            tc.tile_pool(name="pstr", bufs=2, space="PSUM") as pstr_pool,
            tc.tile_pool(name="ps1", bufs=1, space="PSUM") as ps1_pool,
            tc.tile_pool(name="psb", bufs=1, space="PSUM") as psb_pool,
            tc.tile_pool(name="pspool", bufs=1, space="PSUM") as pspool_pool,
        ):
            # persistent tiles
            hT = persist.tile([F, NLOC], f32, tag="hT")
            ad_rep = persist.tile([128, NAUG], f32, tag="ad_rep")
            ad_row = persist.tile([1, NLOC], f32, tag="ad_row")
            adt_all = persist.tile([128, nCH * C], f32, tag="adt_all")
            identity = persist.tile([128, 128], f32, tag="identity")
            ones_col = persist.tile([1, 128], f32, tag="ones_col")
            w_sb = persist.tile([F, F + 1], f32, tag="w_sb")
            asrc_sb = persist.tile([128, F], f32, tag="asrc_sb")
            b_sb = persist.tile([128, F], f32, tag="b_sb")
            ic_out = persist.tile([128, 16 * IC_GROUP * C], f32,
                                  tag="ic_out")
            pay_bufs = [persist.tile([128, C, ROW], f32, tag=f"pay{i}",
                                     name=f"pay{i}")
                        for i in range(KBUF)]
            aidx_sb = persist.tile([128, MA_COLS + MB_COLS], u16,
                                   tag="aidx_sb")

            masks.make_identity(nc, identity[:])
            nc.gpsimd.memset(ones_col[:], 1.0)
            nc.sync.dma_start(aidx_sb[:], aidx_d.ap())
            # zero pad rows of the gather table
            zpad = persist.tile([2, F], f32, tag="zpad")
            nc.gpsimd.memset(zpad[:], 0.0)
            nc.sync.dma_start(h_table.ap()[0:1], zpad[0:1])
            nc.sync.dma_start(h_table.ap()[RTAB - 1:RTAB], zpad[1:2])

            for pb_ in pay_bufs:
                nc.vector.memset(pb_[:, :, F + 1:ROW], 0.0)
            for rep in range(repeat):
              nc.sync.dma_start(hT[:], xT_own.ap())
              for oa in out_augs:
                nc.scalar.dma_start(oa.ap()[:], zrow_d.ap()[:])
              for k in range(3):
                  nc.sync.dma_start(w_sb[:], w_aug_d.ap()[k])
                  nc.sync.dma_start(asrc_sb[:], asrc_d.ap()[k])
                  nc.sync.dma_start(b_sb[:], b_rep_d.ap()[k])

                  # ---- table build: all NPAD nodes, blocked ----
                  # block loads/stores cut HWDGE instruction count; loads on
                  # SP queue, stores on ACT queue to parallelize sequencers
                  def table_block(t0, nt, load_src):
                      lhsT = lhs_pool.tile([128, 4, 128], f32, tag="lhsT")
                      load_src(lhsT, t0, nt)
                      ps = ps_pool.tile([128, 4, F], f32, tag="ps_tab")
                      for i in range(nt):
                          nc.tensor.matmul(ps[:, i], lhsT[:, i], w_sb[:, 0:F],
                                           start=True, stop=True)
                      st = stage_pool.tile([128, 4, F], f32, tag="stage")
                      nc.scalar.activation(st[:, 0:nt], ps[:, 0:nt], AF.Copy)
                      dst = h_table.ap()[1 + 128 * t0:1 + 128 * (t0 + nt)] \
                          .rearrange("(t p) f -> p t f", t=nt)
                      nc.scalar.dma_start(dst, st[:, 0:nt])

                  if "notable" in features:
                      pass
                  elif k == 0:
                      def load0(lhsT, t0, nt):
                          nc.sync.dma_start(
                              lhsT[:, 0:nt],
                              xT_full.ap()[:, 128 * t0:128 * (t0 + nt)]
                              .rearrange("p (t f) -> p t f", t=nt))
                      for blk in range(TTILES // 4):
                          table_block(4 * blk, 4, load0)
                  else:
                      def load1(lhsT, t0, nt):
                          rr, jj = t0 // NTILES, t0 % NTILES
                          nc.sync.dma_start(
                              lhsT[:, 0:nt],
                              cc_out.ap()[rr, :, 128 * jj:128 * (jj + nt)]
                              .rearrange("p (t f) -> p t f", t=nt))
                      for rr in range(NCORES):
                          base = rr * NTILES
                          pos = 0
                          while pos < NTILES:
                              nt = min(4, NTILES - pos)
                              table_block(base + pos, nt, load1)
                              pos += nt

                  # ---- alpha_d of own nodes -> replicated [128, NAUG] ----
                  ad_chunks = []
                  pos = 0
                  while pos < NLOC:
                      sz = min(512, NLOC - pos)
                      ad_chunks.append((pos, sz))
                      pos += sz
                  for pos, sz in ad_chunks:
                      sl = slice(pos, pos + sz)
                      pr = ps1_pool.tile([1, 512], f32, tag="ps_ad")
                      nc.tensor.matmul(pr[:, 0:sz], w_sb[:, F:F + 1],
                                       hT[:, sl], start=True, stop=True)
                      nc.vector.tensor_copy(ad_row[:, sl], pr[:, 0:sz])
                  for pos, sz in ad_chunks:
                      sl = slice(pos, pos + sz)
                      pb = psb_pool.tile([128, 512], f32, tag="ps_bc")
                      nc.tensor.matmul(pb[:, 0:sz], ones_col[:], ad_row[:, sl],
                                       start=True, stop=True)
                      nc.vector.tensor_copy(ad_rep[:, sl], pb[:, 0:sz])
                  nc.vector.memset(ad_rep[:, NLOC:NAUG], BIG_NEG)

                  # ---- per-token alpha_d: 2 big indirect copies + reshape ----
                  if "ic" in features:
                      for c0, gsz in _ic_groups(nA, nB):
                          C_all = gsz * C
                          M = 16 * C_all
                          nc.gpsimd.indirect_copy(
                              ic_out[:, 0:M], ad_rep[:],
                              aidx_sb[:, c0 * C:c0 * C + C_all], True)
                          src_ap = ic_out[:, 0:M].rearrange(
                              "(g o) (kk j) -> g o kk j",
                              g=8, o=16, kk=16, j=C_all)[:, 0]
                          nc.sync.dma_start(
                              adt_all[:, c0 * C:c0 * C + C_all], src_ap)
                  else:
                      nc.vector.memset(adt_all[:], 0.0)

                  # ---- edge phase ----
                  for ci in range(nCH):
                      bank = h_table.ap()[0:BANK] if ci < nA \
                          else h_table.ap()[BANK:RTAB]
                      gsi = edge_pool.tile([128, 2 * (CHUNK // 16)], i16,
                                           tag="gsi")
                      nc.sync.dma_start(gsi[:], gsidx_d.ap()[ci])
                      gi = gsi[:, 0:CHUNK // 16]
                      si = gsi[:, CHUNK // 16:2 * (CHUNK // 16)]

                      gbuf = gb_pool.tile([128, C, F], f32, tag="gbuf")
                      if "gather" in features:
                          nc.gpsimd.dma_gather(gbuf[:], bank, gi, CHUNK,
                                               CHUNK, F, single_packet=False)
                      else:
                          nc.vector.memset(gbuf[:], 0.0)

                      als = edge_pool.tile([128, C], f32, tag="als")
                      if "noedve" in features:
                          nc.vector.memset(als[:], 0.0)
                      else:
                          prod = edge_pool.tile([128, C, F], f32, tag="prod")
                          a_bc = asrc_sb[:].unsqueeze(1).broadcast_to([128, C, F])
                          nc.vector.tensor_tensor(prod[:], gbuf[:], a_bc,
                                                  ALU.mult)
                          nc.vector.tensor_reduce(als[:], prod[:], AX.X, ALU.add)

                      e = edge_pool.tile([128, C], f32, tag="e")
                      nc.vector.tensor_tensor(e[:], als[:],
                                              adt_all[:, ci * C:ci * C + C],
                                              ALU.add)
                      nc.vector.scalar_tensor_tensor(e[:], e[:], NEG_SLOPE, e[:],
                                                     ALU.mult, ALU.max)
                      p = edge_pool.tile([128, C], f32, tag="p")
                      nc.scalar.activation(p[:], e[:], AF.Exp)

                      pay = pay_bufs[ci % KBUF]
                      if "noedve" not in features:
                          p_b = p[:].unsqueeze(2).broadcast_to([128, C, F])
                          nc.vector.tensor_tensor(pay[:, :, 0:F], gbuf[:], p_b,
                                                  ALU.mult)
                          nc.vector.tensor_copy(pay[:, :, F], p[:])
                      if "scatter" in features:
                          nc.gpsimd.dma_scatter_add(
                              out_augs[2 * k + ci % 2].ap()[:], pay[:], si,
                              CHUNK, CHUNK, ROW, single_packet=False)

                  # ---- post-process ----
                  if k == 2:
                      ps_pl = pspool_pool.tile([G, F], f32, tag="ps_pl")
                  for j in range(NTILES):
                      poA = post_pool.tile([128, F + 1], f32, tag="poA")
                      poB = post_pool.tile([128, F + 1], f32, tag="poB")
                      nc.scalar.dma_start(
                          poA[:],
                          out_augs[2 * k].ap()[128 * j:128 * j + 128, 0:F + 1])
                      nc.scalar.dma_start(
                          poB[:],
                          out_augs[2 * k + 1].ap()[128 * j:128 * j + 128, 0:F + 1])
                      po = post_pool.tile([128, F + 1], f32, tag="po")
                      nc.vector.tensor_tensor(po[:], poA[:], poB[:], ALU.add)
                      s_t = post_pool.tile([128, 1], f32, tag="s_t")
                      nc.vector.tensor_scalar_add(s_t[:], po[:, F:F + 1], EPS)
                      r_t = post_pool.tile([128, 1], f32, tag="r_t")
                      nc.vector.reciprocal(r_t[:], s_t[:])
                      h1 = post_pool.tile([128, F], f32, tag="h1")
                      nc.vector.tensor_scalar(h1[:], po[:, 0:F], r_t[:], None,
                                              ALU.mult)
                      nc.vector.tensor_tensor(h1[:], h1[:], b_sb[:], ALU.add)
                      # ELU = relu(x) + expm1(min(x,0))
                      mn = post_pool.tile([128, F], f32, tag="mn")
                      nc.vector.tensor_scalar_min(mn[:], h1[:], 0.0)
                      ex = post_pool.tile([128, F], f32, tag="ex")
                      nc.scalar.activation(ex[:], mn[:], AF.Exp)
                      rl = post_pool.tile([128, F], f32, tag="rl")
                      nc.vector.tensor_scalar_max(rl[:], h1[:], 0.0)
                      ho = post_pool.tile([128, F], f32, tag="ho")
                      nc.vector.scalar_tensor_tensor(ho[:], ex[:], -1.0, rl[:],
                                                     ALU.add, ALU.add)
                      if k < 2:
                          pt = pstr_pool.tile([128, 128], f32, tag="ps_tr")
                          nc.tensor.transpose(pt[:], ho[:], identity[:])
                          nc.vector.tensor_copy(hT[:, 128 * j:128 * j + 128],
                                                pt[:])
                      else:
                          ph = post_pool.tile([128, G], f32, tag="ph")
                          nc.sync.dma_start(
                              ph[:], phot_d.ap()[128 * j:128 * j + 128])
                          nc.tensor.matmul(ps_pl[:], ph[:], ho[:],
                                           start=(j == 0),
                                           stop=(j == NTILES - 1))

                  if k < 2:
                      nc.sync.dma_start(cc_in.ap()[:], hT[:])
                      if "cc" in features:
                          nc.gpsimd.collective_compute(
                              "AllGather", mybir.AluOpType.bypass,
                              replica_groups=[list(range(NCORES))],
                              ins=[cc_in.ap().opt()], outs=[cc_out.ap().opt()])
                      else:
                          for rr in range(NCORES):
                              nc.sync.dma_start(cc_out.ap()[rr], cc_in.ap()[:])
                  else:
                      pl_sb = post_pool.tile([G, F], f32, tag="pl_sb")
                      nc.vector.tensor_copy(pl_sb[:], ps_pl[:])
                      nc.sync.dma_start(pool_out.ap()[:], pl_sb[:])

    nc.compile()
    return nc


# ---------------- entry point ----------------

LAST_EXEC_NS = None
LAST_RES = None


def kernel(x, edge_index, batch,
           W1, a_src1, a_dst1, b1,
           W2, a_src2, a_dst2, b2,
           W3, a_src3, a_dst3, b3):
    global LAST_EXEC_NS, LAST_RES
    x = np.asarray(x, np.float32)
    edge_index = np.asarray(edge_index)
    batch = np.asarray(batch)
    Ws = [np.asarray(W1, np.float32), np.asarray(W2, np.float32),
          np.asarray(W3, np.float32)]
    asrcs = [np.asarray(a_src1, np.float32), np.asarray(a_src2, np.float32),
             np.asarray(a_src3, np.float32)]
    adsts = [np.asarray(a_dst1, np.float32), np.asarray(a_dst2, np.float32),
             np.asarray(a_dst3, np.float32)]
    bs = [np.asarray(b1, np.float32), np.asarray(b2, np.float32),
          np.asarray(b3, np.float32)]

    in_maps, nA, nB, counts = _prep_inputs(x, edge_index, batch, Ws, asrcs,
                                           adsts, bs)

    from concourse.bass_utils import run_bass_kernel_spmd
    nc = _build_program(nA, nB)
    res = run_bass_kernel_spmd(nc, in_maps, core_ids=list(range(NCORES)))
    LAST_EXEC_NS = res.exec_time_ns
    LAST_RES = res
    total = np.zeros((G, F), np.float32)
    for r in range(NCORES):
        total += res.results[r]["pool_part"]
    out = total / np.maximum(counts, 1.0)[:, None]
    return out.astype(np.float32)

